# revision 36
# baseline (speedup 1.0000x reference)
"""CrossATT kernel for Trainium2 (Bass/Tile), data-parallel over batch on 8 cores.

Math (per batch b):
    S = x_cont @ x_ques^T            # [C, Q]
    A = softmax(S, axis=-1)          # over q
    c2q = A @ x_ques                 # [C, D]
    out = c2q @ W1 + x_cont @ W0     # [C, D]

Device-side formulation works fully transposed so the TensorE contraction
axis is always on partitions and softmax needs no on-chip transposes.
W1 is folded into x_ques on the host (QW = x_ques @ W1) and the W0 term
(x_cont @ W0, no attention dependence) is added on the host.  The softmax
normalization is ALSO finished on the host: per 512-column c-block the
device ships the unnormalized OT[e, c] = sum_q QW[q, e] E[q, c] plus the
per-partition partial sums part[p, c] = sum_k E[128*k + p, c]; the host
reduces part over p for the denominator s[c] and divides.  This removes
the reciprocal, the cross-partition broadcast, the s-matmul and the
normalize-multiply from the device entirely.

Per-block engine assignment, balanced against the PE's ~1.73us of matmul
streaming (measured steady state 1.72-1.75us/block):
    TensorE : MM1 x4 (fp16, N=512) + MM2 x4 (bf16 rhs, accumulate)
    ScalarE : exp of q-chunks 0,1 (one ACTIVATE, PSUM f32 -> bf16) and the
              PSUM->SBUF copy of MM2's output
    VectorE : q-chunks 2,3 via a one-instruction Schraudolph exp2
              (int16(S*128/ln2 + b) bitcast as bf16; ~3% rel err on HALF
              the attention weights, softmax self-normalization cancels
              most of it - measured end-to-end 9.9e-3 vs the 2e-2 gate)
              plus the first fold add
    GpSimd  : second fold add (SBUF-only engine, otherwise idle)
All exp on ScalarE alone would cost 2.2us/block on that one engine and
bind the pipeline below the PE's rate.

Startup is dominated by DGE-ring latency: a descriptor issued at ~7.2us
delivers its first packets ~2.4us later, so the first real matmul cannot
start before ~10.5us.  Mitigations: ScalarE's ring carries ONLY the qt0
descriptor (the ACT-table load + first exp must run by ~11.8us; qw/qt1/qw1
descriptors ride the GpSimd ring, XT the sync ring), and a ~36-matmul
N=64 prewarm burst keeps the PE's HAM activity window continuously busy
from ~7us until the gating DMA lands, so the 2.4GHz clock engages at
~11-14us instead of ~21us.  Stores are per-block (256KB) so the tail only
drains one block after the last matmul.
"""

import os

import numpy as np

import concourse.bass as bass
import concourse.mybir as mybir
import concourse.tile as tile
from concourse import bacc
from concourse.bass_utils import run_bass_kernel_spmd

B, C_LEN, Q_LEN, D = 16, 4096, 512, 128
NCORES = 8
BPC = B // NCORES          # batches per core
CB = 512                   # c-block width (PSUM bank / max f32 moving width)
NBLK = C_LEN // CB         # 8 blocks per batch
NQ = Q_LEN // 128          # 4 q-chunks

F32 = mybir.dt.float32
F16 = mybir.dt.float16
BF16 = mybir.dt.bfloat16
I16 = mybir.dt.int16

# exp(x) ~= bitcast_bf16(int16(x * 128/ln2 + SCHR_B)); SCHR_B calibrated
# against np.exp for min RMS relative error assuming round-to-nearest
# f32->int16 conversion (host-validated end-to-end at ~9e-3 vs 2e-2 gate).
SCHR_A = 128.0 / float(np.log(2.0))
SCHR_B = float(os.environ.get("SCHR_B", "16248.5"))

EXPSPLIT = int(os.environ.get("EXPSPLIT", "1"))
PREWARM = int(os.environ.get("PREWARM", "20"))

_CACHE = {}


def _build():
    nc = bacc.Bacc("TRN2", target_bir_lowering=False, debug=False, num_devices=NCORES)

    XT = nc.declare_dram_parameter("XT", [BPC, D, C_LEN], F16, isOutput=False)
    QT = nc.declare_dram_parameter("QT", [BPC, D, Q_LEN], F16, isOutput=False)
    QW = nc.declare_dram_parameter("QW", [BPC, 128, NQ, D], BF16, isOutput=False)
    # OUT[b][p, jp, u, kind, c]: kind 0 = OT rows (e), kind 1 = part rows
    # (the folded E partial sums); the host reduces part over partitions to
    # get the softmax denominator.
    OUT = nc.declare_dram_parameter(
        "OUT", [BPC, 128, NBLK // 2, 2, 2, CB], BF16, isOutput=True
    )

    with tile.TileContext(nc) as tc:
        with (
            tc.tile_pool(name="const", bufs=1) as const,
            tc.tile_pool(name="xt", bufs=1) as xtp,
            tc.tile_pool(name="e", bufs=3) as ep,
            tc.tile_pool(name="q0", bufs=3) as q0p,
            tc.tile_pool(name="res", bufs=3) as resp,
            tc.tile_pool(name="ps_st", bufs=3, space="PSUM") as ps_st,
            tc.tile_pool(name="ps_o", bufs=2, space="PSUM") as ps_o,
        ):
            # Preamble DMA order is latency-critical: the first MM1 needs
            # only QT[0] (one 128KB descriptor, scalar ring) and the first
            # XT half-pair (sync ring).  Everything else avoids the scalar
            # ring entirely: the descriptor issue costs ~650ns of ScalarE
            # time each, and ScalarE must be free for the ACT-table load +
            # the first exp by ~9us or the whole pipeline ramp stalls on
            # PSUM slot reuse.  QW0 rides the idle VectorE ring, batch 1's
            # QT1/QW1 (not needed until ~block 8) ride the GpSimd ring.
            wsrc = const.tile([128, 256], F16, name="warm")
            nc.gpsimd.memset(wsrc, 0.25)

            qt_sb = []
            qt0 = const.tile([D, Q_LEN], F16, name="qt0")
            nc.scalar.dma_start(out=qt0, in_=QT[0])
            qt_sb.append(qt0)

            def xt_tile(b, jp):
                return xtp.tile([D, 2, CB], F16, tag=f"xt{b}_{jp}", name=f"xt{b}_{jp}")

            xt_sb = {}
            t00 = xt_tile(0, 0)
            nc.sync.dma_start(
                out=t00[:, 0, :], in_=XT[0][:, bass.ts(0, CB)]
            )
            nc.sync.dma_start(out=t00[:, 1, :], in_=XT[0][:, bass.ts(1, CB)])
            xt_sb[(0, 0)] = t00

            qw_sb = []
            qw0 = const.tile([128, NQ, D], BF16, name="qw0")
            nc.gpsimd.dma_start(out=qw0, in_=QW[0])
            qw_sb.append(qw0)

            for jp in range(1, NBLK // 2):
                t = xt_tile(0, jp)
                nc.sync.dma_start(
                    out=t,
                    in_=XT[0][:, bass.ts(jp, 2 * CB)].rearrange("d (u c) -> d u c", u=2),
                )
                xt_sb[(0, jp)] = t

            qt1 = const.tile([D, Q_LEN], F16, name="qt1")
            nc.gpsimd.dma_start(out=qt1, in_=QT[1])
            qt_sb.append(qt1)
            qw1 = const.tile([128, NQ, D], BF16, name="qw1")
            nc.gpsimd.dma_start(out=qw1, in_=QW[1])
            qw_sb.append(qw1)

            for jp in range(NBLK // 2):
                t = xt_tile(1, jp)
                nc.sync.dma_start(
                    out=t,
                    in_=XT[1][:, bass.ts(jp, 2 * CB)].rearrange("d (u c) -> d u c", u=2),
                )
                xt_sb[(1, jp)] = t

            # HAM prewarm: a burst of cheap N=64 matmuls keeps the PE's
            # activity window busy while the gating DMAs land, so the real
            # stream hits the 2.4GHz clock ~2-3us sooner.  They recycle the
            # ps_st ring before any real MM1 touches it.
            if PREWARM:
                for w in range(PREWARM):
                    st = ps_st.tile([128, 2, CB], F32, tag="st")
                    nc.tensor.matmul(
                        out=st[0:64, 0, 0:64],
                        lhsT=wsrc[:, 0:64],
                        rhs=wsrc[:, 0:64],
                        start=True,
                        stop=True,
                    )

            blocks = []
            for b in range(BPC):
                for jp in range(NBLK // 2):
                    for u in range(2):
                        blocks.append((b, jp, u))
            n = len(blocks)

            st1 = {}  # j -> (e_t, q0_t, res_t, blk)

            def stage1(j):
                b, jp, u = blocks[j]
                if u == 0:
                    st1["res"] = resp.tile(
                        [128, 2, 2, CB], BF16, tag="res", name=f"res{j}"
                    )
                res_t = st1["res"]
                xt_blk = xt_sb[(b, jp)][:, u, :]

                stA = ps_st.tile([128, 2, CB], F32, tag="st", name=f"stA{j}")
                for k in (0, 1):
                    nc.tensor.matmul(
                        out=stA[:, k, :],
                        lhsT=qt_sb[b][:, bass.ts(k, 128)],
                        rhs=xt_blk,
                        start=True,
                        stop=True,
                    )
                stB = ps_st.tile([128, 2, CB], F32, tag="st", name=f"stB{j}")
                for k in (0, 1):
                    nc.tensor.matmul(
                        out=stB[:, k, :],
                        lhsT=qt_sb[b][:, bass.ts(2 + k, 128)],
                        rhs=xt_blk,
                        start=True,
                        stop=True,
                    )

                e_t = ep.tile([128, NQ, CB], BF16, tag="e", name=f"e{j}")
                # chunks 0,1: exact exp on ScalarE (PSUM f32 -> SBUF bf16)
                nc.scalar.activation(
                    out=e_t[:, 0:2, :],
                    in_=stA,
                    func=mybir.ActivationFunctionType.Exp,
                )
                with nc.allow_low_precision(
                    reason="Schraudolph exp2 in bf16 for half the attention "
                    "weights; softmax self-normalization cancels most of the "
                    "~3% per-weight error (end-to-end ~1e-2 vs 2e-2 gate)"
                ):
                    if EXPSPLIT:
                        # chunks 2,3: one-instruction exp approx on VectorE
                        nc.vector.tensor_scalar(
                            e_t.bitcast(I16)[:, 2:4, :],
                            stB,
                            SCHR_A,
                            SCHR_B,
                            mybir.AluOpType.mult,
                            mybir.AluOpType.add,
                        )
                    else:
                        nc.scalar.activation(
                            out=e_t[:, 2:4, :],
                            in_=stB,
                            func=mybir.ActivationFunctionType.Exp,
                        )
                    # first fold: q0[:, i, :] = E_i + E_{i+2}
                    q0_t = q0p.tile([128, 2, CB], BF16, tag="q0", name=f"q0{j}")
                    nc.vector.tensor_add(
                        out=q0_t, in0=e_t[:, 0:2, :], in1=e_t[:, 2:4, :]
                    )
                    if j >= n - 2:
                        # last pair: finish the fold on VectorE so the drain
                        # doesn't serialize on GpSimd's slower adds
                        nc.vector.tensor_add(
                            out=res_t[:, u, 1, :],
                            in0=q0_t[:, 0, :],
                            in1=q0_t[:, 1, :],
                        )
                st1[j] = (e_t, q0_t, res_t, (b, jp, u))

            def stage2(j):
                e_t, q0_t, res_t, (b, jp, u) = st1[j]
                # second fold into the shipped tile: part = sum of all 4
                # q-chunks.  GpSimd (SBUF-only, otherwise idle) takes it,
                # except for the last pair where its slow adds would
                # serialize the drain - VectorE is free by then.
                if j < n - 2:
                    with nc.allow_low_precision(
                        reason="bf16 partial softmax sums; host reduces in f32"
                    ):
                        nc.gpsimd.tensor_add(
                            out=res_t[:, u, 1, :],
                            in0=q0_t[:, 0, :],
                            in1=q0_t[:, 1, :],
                        )
                # MM2: OT = QW^T E (unnormalized c2q @ W1, transposed)
                o_ps = ps_o.tile([D, CB], F32, tag="o", name=f"ops{j}")
                for k in range(NQ):
                    nc.tensor.matmul(
                        out=o_ps,
                        lhsT=qw_sb[b][:, k, :],
                        rhs=e_t[:, k, :],
                        start=(k == 0),
                        stop=(k == NQ - 1),
                    )
                # PSUM -> SBUF (bf16) on ScalarE; host divides by s
                with nc.allow_low_precision(reason="bf16 OT ship; 2e-2 gate"):
                    nc.scalar.copy(out=res_t[:, u, 0, :], in_=o_ps)

            def stage3(j):
                _, _, res_t, (b, jp, u) = st1.pop(j)
                # per-block stores: each 256KB transfer starts as soon as its
                # block's OT copy and part fold land, so the tail only drains
                # one block's worth of data after the last matmul.  The last
                # PAIR is further split per kind-plane and spread over the
                # scalar/gpsimd/sync rings (all idle by then): four parallel
                # 128KB transfers instead of two serialized 256KB ones -
                # each plane also starts as soon as ITS producer (ACT copy
                # or VectorE fold) lands, not both.
                if j == n - 2:
                    nc.scalar.dma_start(
                        out=OUT[b][:, jp, u, 0], in_=res_t[:, u, 0, :]
                    )
                    nc.gpsimd.dma_start(
                        out=OUT[b][:, jp, u, 1], in_=res_t[:, u, 1, :]
                    )
                elif j == n - 1:
                    nc.sync.dma_start(
                        out=OUT[b][:, jp, u, 0], in_=res_t[:, u, 0, :]
                    )
                    nc.gpsimd.dma_start(
                        out=OUT[b][:, jp, u, 1], in_=res_t[:, u, 1, :]
                    )
                else:
                    nc.sync.dma_start(out=OUT[b][:, jp, u], in_=res_t[:, u])

            for j in range(n + 2):
                if j < n:
                    stage1(j)
                if 0 <= j - 1 < n:
                    stage2(j - 1)
                if 0 <= j - 2:
                    stage3(j - 2)

    nc.compile()
    return nc


def _prep_inputs(x_cont, x_ques, W1):
    bf16 = mybir.dt.np(BF16)
    xt = np.ascontiguousarray(
        x_cont.transpose(0, 2, 1), dtype=np.float16
    )                                                          # [B, D, C] f16
    qt = np.ascontiguousarray(
        x_ques.transpose(0, 2, 1), dtype=np.float16
    )                                                          # [B, D, Q] f16
    qw = np.matmul(x_ques, W1).astype(bf16)                    # [B, Q, D] bf16
    # device-side stationary layout: [B, 128, NQ, D], contiguous DMA lines
    qw = np.ascontiguousarray(
        qw.reshape(B, Q_LEN // 128, 128, D).transpose(0, 2, 1, 3)
    )
    return xt, qt, qw


def _gather(results, x_cont, W0):
    """Host-side finish: unpack OUT, reduce the E planes -> s, normalize,
    add the W0 term."""
    out = np.matmul(x_cont, W0)  # [B, C, D] - attention-free term
    for i in range(len(results)):
        o = np.asarray(results[i]["OUT"]).astype(np.float32)
        # [BPC, 128, NBLK//2, 2, 2, CB] -> [BPC, 128, 2, C_LEN]
        o = o.transpose(0, 1, 4, 2, 3, 5).reshape(BPC, 128, 2, C_LEN)
        ot = o[:, :, 0]                            # [BPC, e, c]
        s = o[:, :, 1].sum(axis=1)                 # [BPC, c]
        att = ot / s[:, None, :]                   # normalize
        out[i * BPC : (i + 1) * BPC] += att.transpose(0, 2, 1)
    return out


def kernel(x_cont, x_ques, c_mask, q_mask, W0, W1):
    x_cont = np.ascontiguousarray(x_cont, dtype=np.float32)
    x_ques = np.ascontiguousarray(x_ques, dtype=np.float32)
    W0 = np.ascontiguousarray(W0, dtype=np.float32)
    W1 = np.ascontiguousarray(W1, dtype=np.float32)

    if "nc" not in _CACHE:
        _CACHE["nc"] = _build()
    nc = _CACHE["nc"]

    xt, qt, qw = _prep_inputs(x_cont, x_ques, W1)

    in_maps = []
    for i in range(NCORES):
        sl = slice(i * BPC, (i + 1) * BPC)
        in_maps.append({"XT": xt[sl], "QT": qt[sl], "QW": qw[sl]})

    res = run_bass_kernel_spmd(nc, in_maps, core_ids=list(range(NCORES)))

    return _gather([res.results[i] for i in range(NCORES)], x_cont, W0)


# revision 37
# speedup vs baseline: 1.0012x; 1.0012x over previous
"""CrossATT kernel for Trainium2 (Bass/Tile), data-parallel over batch on 8 cores.

Math (per batch b):
    S = x_cont @ x_ques^T            # [C, Q]
    A = softmax(S, axis=-1)          # over q
    c2q = A @ x_ques                 # [C, D]
    out = c2q @ W1 + x_cont @ W0     # [C, D]

Device-side formulation works fully transposed so the TensorE contraction
axis is always on partitions and softmax needs no on-chip transposes.
W1 is folded into x_ques on the host (QW = x_ques @ W1) and the W0 term
(x_cont @ W0, no attention dependence) is added on the host.  The softmax
normalization is ALSO finished on the host: per 512-column c-block the
device ships the unnormalized OT[e, c] = sum_q QW[q, e] E[q, c] plus the
per-partition partial sums part[p, c] = sum_k E[128*k + p, c]; the host
reduces part over p for the denominator s[c] and divides.  This removes
the reciprocal, the cross-partition broadcast, the s-matmul and the
normalize-multiply from the device entirely.

Per-block engine assignment, balanced against the PE's ~1.73us of matmul
streaming (measured steady state 1.72-1.75us/block):
    TensorE : MM1 x4 (fp16, N=512) + MM2 x4 (bf16 rhs, accumulate)
    ScalarE : exp of q-chunks 0,1 (one ACTIVATE, PSUM f32 -> bf16) and the
              PSUM->SBUF copy of MM2's output
    VectorE : q-chunks 2,3 via a one-instruction Schraudolph exp2
              (int16(S*128/ln2 + b) bitcast as bf16; ~3% rel err on HALF
              the attention weights, softmax self-normalization cancels
              most of it - measured end-to-end 9.9e-3 vs the 2e-2 gate)
              plus the first fold add
    GpSimd  : second fold add (SBUF-only engine, otherwise idle)
All exp on ScalarE alone would cost 2.2us/block on that one engine and
bind the pipeline below the PE's rate.

Startup is dominated by DGE-ring latency: a descriptor issued at ~7.2us
delivers its first packets ~2.4us later, so the first real matmul cannot
start before ~10.5us.  Mitigations: ScalarE's ring carries ONLY the qt0
descriptor (the ACT-table load + first exp must run by ~11.8us; qw/qt1/qw1
descriptors ride the GpSimd ring, XT the sync ring), and a ~36-matmul
N=64 prewarm burst keeps the PE's HAM activity window continuously busy
from ~7us until the gating DMA lands, so the 2.4GHz clock engages at
~11-14us instead of ~21us.  Stores are per-block (256KB) so the tail only
drains one block after the last matmul.
"""

import os

import numpy as np

import concourse.bass as bass
import concourse.mybir as mybir
import concourse.tile as tile
from concourse import bacc
from concourse.bass_utils import run_bass_kernel_spmd

B, C_LEN, Q_LEN, D = 16, 4096, 512, 128
NCORES = 8
BPC = B // NCORES          # batches per core
CB = 512                   # c-block width (PSUM bank / max f32 moving width)
NBLK = C_LEN // CB         # 8 blocks per batch
NQ = Q_LEN // 128          # 4 q-chunks

F32 = mybir.dt.float32
F16 = mybir.dt.float16
BF16 = mybir.dt.bfloat16
I16 = mybir.dt.int16

# exp(x) ~= bitcast_bf16(int16(x * 128/ln2 + SCHR_B)); SCHR_B calibrated
# against np.exp for min RMS relative error assuming round-to-nearest
# f32->int16 conversion (host-validated end-to-end at ~9e-3 vs 2e-2 gate).
SCHR_A = 128.0 / float(np.log(2.0))
SCHR_B = float(os.environ.get("SCHR_B", "16248.5"))

EXPSPLIT = int(os.environ.get("EXPSPLIT", "1"))
PREWARM = int(os.environ.get("PREWARM", "20"))

_CACHE = {}


def _build():
    nc = bacc.Bacc("TRN2", target_bir_lowering=False, debug=False, num_devices=NCORES)

    XT = nc.declare_dram_parameter("XT", [BPC, D, C_LEN], F16, isOutput=False)
    QT = nc.declare_dram_parameter("QT", [BPC, D, Q_LEN], F16, isOutput=False)
    QW = nc.declare_dram_parameter("QW", [BPC, 128, NQ, D], BF16, isOutput=False)
    # OUT[b][p, jp, u, kind, c]: kind 0 = OT rows (e), kind 1 = part rows
    # (the folded E partial sums); the host reduces part over partitions to
    # get the softmax denominator.
    OUT = nc.declare_dram_parameter(
        "OUT", [BPC, 128, NBLK // 2, 2, 2, CB], BF16, isOutput=True
    )

    with tile.TileContext(nc) as tc:
        with (
            tc.tile_pool(name="const", bufs=1) as const,
            tc.tile_pool(name="xt", bufs=1) as xtp,
            tc.tile_pool(name="e", bufs=3) as ep,
            tc.tile_pool(name="q0", bufs=3) as q0p,
            tc.tile_pool(name="res", bufs=3) as resp,
            tc.tile_pool(name="ps_st", bufs=3, space="PSUM") as ps_st,
            tc.tile_pool(name="ps_o", bufs=2, space="PSUM") as ps_o,
        ):
            # Preamble DMA order is latency-critical: the first MM1 needs
            # only QT[0] (one 128KB descriptor, scalar ring) and the first
            # XT half-pair (sync ring).  Everything else avoids the scalar
            # ring entirely: the descriptor issue costs ~650ns of ScalarE
            # time each, and ScalarE must be free for the ACT-table load +
            # the first exp by ~9us or the whole pipeline ramp stalls on
            # PSUM slot reuse.  QW0 rides the idle VectorE ring, batch 1's
            # QT1/QW1 (not needed until ~block 8) ride the GpSimd ring.
            wsrc = const.tile([128, 256], F16, name="warm")
            nc.gpsimd.memset(wsrc, 0.25)

            qt_sb = []
            qt0 = const.tile([D, Q_LEN], F16, name="qt0")
            nc.scalar.dma_start(out=qt0, in_=QT[0])
            qt_sb.append(qt0)

            def xt_tile(b, jp):
                return xtp.tile([D, 2, CB], F16, tag=f"xt{b}_{jp}", name=f"xt{b}_{jp}")

            xt_sb = {}
            t00 = xt_tile(0, 0)
            nc.sync.dma_start(
                out=t00[:, 0, :], in_=XT[0][:, bass.ts(0, CB)]
            )
            nc.sync.dma_start(out=t00[:, 1, :], in_=XT[0][:, bass.ts(1, CB)])
            xt_sb[(0, 0)] = t00

            qw_sb = []
            qw0 = const.tile([128, NQ, D], BF16, name="qw0")
            nc.gpsimd.dma_start(out=qw0, in_=QW[0])
            qw_sb.append(qw0)

            for jp in range(1, NBLK // 2):
                t = xt_tile(0, jp)
                nc.sync.dma_start(
                    out=t,
                    in_=XT[0][:, bass.ts(jp, 2 * CB)].rearrange("d (u c) -> d u c", u=2),
                )
                xt_sb[(0, jp)] = t

            qt1 = const.tile([D, Q_LEN], F16, name="qt1")
            nc.gpsimd.dma_start(out=qt1, in_=QT[1])
            qt_sb.append(qt1)
            qw1 = const.tile([128, NQ, D], BF16, name="qw1")
            nc.gpsimd.dma_start(out=qw1, in_=QW[1])
            qw_sb.append(qw1)

            for jp in range(NBLK // 2):
                t = xt_tile(1, jp)
                nc.sync.dma_start(
                    out=t,
                    in_=XT[1][:, bass.ts(jp, 2 * CB)].rearrange("d (u c) -> d u c", u=2),
                )
                xt_sb[(1, jp)] = t

            # HAM prewarm: a burst of cheap N=64 matmuls keeps the PE's
            # activity window busy while the gating DMAs land, so the real
            # stream hits the 2.4GHz clock ~2-3us sooner.  They recycle the
            # ps_st ring before any real MM1 touches it.
            if PREWARM:
                for w in range(PREWARM):
                    st = ps_st.tile([128, 2, CB], F32, tag="st")
                    nc.tensor.matmul(
                        out=st[0:64, 0, 0:64],
                        lhsT=wsrc[:, 0:64],
                        rhs=wsrc[:, 0:64],
                        start=True,
                        stop=True,
                    )

            blocks = []
            for b in range(BPC):
                for jp in range(NBLK // 2):
                    for u in range(2):
                        blocks.append((b, jp, u))
            n = len(blocks)

            st1 = {}  # j -> (e_t, q0_t, res_t, blk)

            def stage1(j):
                b, jp, u = blocks[j]
                if u == 0:
                    st1["res"] = resp.tile(
                        [128, 2, 2, CB], BF16, tag="res", name=f"res{j}"
                    )
                res_t = st1["res"]
                xt_blk = xt_sb[(b, jp)][:, u, :]

                stA = ps_st.tile([128, 2, CB], F32, tag="st", name=f"stA{j}")
                for k in (0, 1):
                    nc.tensor.matmul(
                        out=stA[:, k, :],
                        lhsT=qt_sb[b][:, bass.ts(k, 128)],
                        rhs=xt_blk,
                        start=True,
                        stop=True,
                    )
                stB = ps_st.tile([128, 2, CB], F32, tag="st", name=f"stB{j}")
                for k in (0, 1):
                    nc.tensor.matmul(
                        out=stB[:, k, :],
                        lhsT=qt_sb[b][:, bass.ts(2 + k, 128)],
                        rhs=xt_blk,
                        start=True,
                        stop=True,
                    )

                e_t = ep.tile([128, NQ, CB], BF16, tag="e", name=f"e{j}")
                # chunks 0,1: exact exp on ScalarE (PSUM f32 -> SBUF bf16)
                nc.scalar.activation(
                    out=e_t[:, 0:2, :],
                    in_=stA,
                    func=mybir.ActivationFunctionType.Exp,
                )
                with nc.allow_low_precision(
                    reason="Schraudolph exp2 in bf16 for half the attention "
                    "weights; softmax self-normalization cancels most of the "
                    "~3% per-weight error (end-to-end ~1e-2 vs 2e-2 gate)"
                ):
                    if EXPSPLIT:
                        # chunks 2,3: one-instruction exp approx on VectorE
                        nc.vector.tensor_scalar(
                            e_t.bitcast(I16)[:, 2:4, :],
                            stB,
                            SCHR_A,
                            SCHR_B,
                            mybir.AluOpType.mult,
                            mybir.AluOpType.add,
                        )
                    else:
                        nc.scalar.activation(
                            out=e_t[:, 2:4, :],
                            in_=stB,
                            func=mybir.ActivationFunctionType.Exp,
                        )
                    # first fold: q0[:, i, :] = E_i + E_{i+2}
                    q0_t = q0p.tile([128, 2, CB], BF16, tag="q0", name=f"q0{j}")
                    nc.vector.tensor_add(
                        out=q0_t, in0=e_t[:, 0:2, :], in1=e_t[:, 2:4, :]
                    )
                    if j >= n - 2:
                        # last pair: finish the fold on VectorE so the drain
                        # doesn't serialize on GpSimd's slower adds
                        nc.vector.tensor_add(
                            out=res_t[:, u, 1, :],
                            in0=q0_t[:, 0, :],
                            in1=q0_t[:, 1, :],
                        )
                st1[j] = (e_t, q0_t, res_t, (b, jp, u))

            def stage2(j):
                e_t, q0_t, res_t, (b, jp, u) = st1[j]
                # second fold into the shipped tile: part = sum of all 4
                # q-chunks.  GpSimd (SBUF-only, otherwise idle) takes it,
                # except for the last pair where its slow adds would
                # serialize the drain - VectorE is free by then.
                if j < n - 2:
                    with nc.allow_low_precision(
                        reason="bf16 partial softmax sums; host reduces in f32"
                    ):
                        nc.gpsimd.tensor_add(
                            out=res_t[:, u, 1, :],
                            in0=q0_t[:, 0, :],
                            in1=q0_t[:, 1, :],
                        )
                # MM2: OT = QW^T E (unnormalized c2q @ W1, transposed)
                o_ps = ps_o.tile([D, CB], F32, tag="o", name=f"ops{j}")
                for k in range(NQ):
                    nc.tensor.matmul(
                        out=o_ps,
                        lhsT=qw_sb[b][:, k, :],
                        rhs=e_t[:, k, :],
                        start=(k == 0),
                        stop=(k == NQ - 1),
                    )
                # PSUM -> SBUF (bf16) on ScalarE; host divides by s
                with nc.allow_low_precision(reason="bf16 OT ship; 2e-2 gate"):
                    nc.scalar.copy(out=res_t[:, u, 0, :], in_=o_ps)

            def stage3(j):
                _, _, res_t, (b, jp, u) = st1.pop(j)
                # per-block stores: each 256KB transfer starts as soon as its
                # block's OT copy and part fold land, so the tail only drains
                # one block's worth of data after the last matmul.  The
                # second-to-last store rides the (by then idle) scalar ring
                # so the final two transfers stream on parallel DMA queues
                # instead of serializing behind each other on the sync ring.
                eng = nc.scalar if j == n - 2 else nc.sync
                eng.dma_start(out=OUT[b][:, jp, u], in_=res_t[:, u])

            for j in range(n + 2):
                if j < n:
                    stage1(j)
                if 0 <= j - 1 < n:
                    stage2(j - 1)
                if 0 <= j - 2:
                    stage3(j - 2)

    nc.compile()
    return nc


def _prep_inputs(x_cont, x_ques, W1):
    bf16 = mybir.dt.np(BF16)
    xt = np.ascontiguousarray(
        x_cont.transpose(0, 2, 1), dtype=np.float16
    )                                                          # [B, D, C] f16
    qt = np.ascontiguousarray(
        x_ques.transpose(0, 2, 1), dtype=np.float16
    )                                                          # [B, D, Q] f16
    qw = np.matmul(x_ques, W1).astype(bf16)                    # [B, Q, D] bf16
    # device-side stationary layout: [B, 128, NQ, D], contiguous DMA lines
    qw = np.ascontiguousarray(
        qw.reshape(B, Q_LEN // 128, 128, D).transpose(0, 2, 1, 3)
    )
    return xt, qt, qw


def _gather(results, x_cont, W0):
    """Host-side finish: unpack OUT, reduce the E planes -> s, normalize,
    add the W0 term."""
    out = np.matmul(x_cont, W0)  # [B, C, D] - attention-free term
    for i in range(len(results)):
        o = np.asarray(results[i]["OUT"]).astype(np.float32)
        # [BPC, 128, NBLK//2, 2, 2, CB] -> [BPC, 128, 2, C_LEN]
        o = o.transpose(0, 1, 4, 2, 3, 5).reshape(BPC, 128, 2, C_LEN)
        ot = o[:, :, 0]                            # [BPC, e, c]
        s = o[:, :, 1].sum(axis=1)                 # [BPC, c]
        att = ot / s[:, None, :]                   # normalize
        out[i * BPC : (i + 1) * BPC] += att.transpose(0, 2, 1)
    return out


def kernel(x_cont, x_ques, c_mask, q_mask, W0, W1):
    x_cont = np.ascontiguousarray(x_cont, dtype=np.float32)
    x_ques = np.ascontiguousarray(x_ques, dtype=np.float32)
    W0 = np.ascontiguousarray(W0, dtype=np.float32)
    W1 = np.ascontiguousarray(W1, dtype=np.float32)

    if "nc" not in _CACHE:
        _CACHE["nc"] = _build()
    nc = _CACHE["nc"]

    xt, qt, qw = _prep_inputs(x_cont, x_ques, W1)

    in_maps = []
    for i in range(NCORES):
        sl = slice(i * BPC, (i + 1) * BPC)
        in_maps.append({"XT": xt[sl], "QT": qt[sl], "QW": qw[sl]})

    res = run_bass_kernel_spmd(nc, in_maps, core_ids=list(range(NCORES)))

    return _gather([res.results[i] for i in range(NCORES)], x_cont, W0)
